# revision 27
# baseline (speedup 1.0000x reference)
"""Trainium2 Bass kernel for the two-stage DAN/MoVe attention module.

Computation (per batch b, C=128 channels):
  Stage 1:  S  = skT.T @ q1 / sqrt(C);  P  = softmax_k(S);  newV = sv @ P
  Stage 2:  S2 = mK.T @ qq / sqrt(C);   P2 = softmax_k2(S2); out = newV @ P2

Sharding: 8 cores = 2 batches x 4 lanes. Stage 1 splits the 24000 support
keys 4 ways (47 key tiles each); stage 2 splits the 14400 frame-query
columns 4 ways (3600 each). Two SPMD launches; the host reduces the
stage-1 partial sums, normalizes, and transposes between launches.

All big matmuls run in bf16 (both operands; fp32 PSUM accumulate).
Softmax skips max-subtraction (scores ~N(0,1); exp cannot overflow).
Zero-padded key rows contribute exactly exp(0)=1 each to the stage-1
column sums; the host subtracts that constant instead of masking on
device. Column sums come from DVE-accumulated exp tiles collapsed and
all-reduced across partitions on the (otherwise idle) gpsimd engine --
no tensor-engine csum matmuls at all, which frees a PSUM bank for a
fourth score buffer.

Stage 1 primes all four score PSUM banks, then emits each lookahead
score AFTER the value pack whose exp freed that bank: the in-order
tensor queue then never stalls on the scalar engine's exp chain (the
baseline lost ~21us to 58 such stalls) including across chunk
boundaries. Chunk widths are [512,400,400,288]: the wide first chunk
slows the key-tile sweep while the fused sv|skT slices stream in, and
no chunk is narrow enough to become LDWEIGHTS/sequencer-bound. The
last chunk's final 8 key tiles run value-major so the four
accumulators finish staggered and their evacuation DMAs overlap the
remaining sweeps instead of bunching on the launch tail.

Stage 2 loads newV^T in a partition-major fused layout (one SBUF tile,
long-descriptor DMAs, like stage 1's fus) with the DMA order matched
to first use: mk tile 0 and the first qq chunk gate the first matmuls,
the early nvt tiles gate the first output matmuls.  The baseline's
1028B-descriptor nvte loads landed ~9us in, stalling the tensor queue
3.3us and triggering a HAM re-throttle; filler matmuls now bridge the
unavoidable nvt wait so the clock never drops.  Uniform 450-wide
chunks avoid the sequencer-bound 240-wide tail chunk.  All PSUM
evacuations ride the DVE so the scalar engine runs only the exp chain.

Each launch opens with throwaway matmuls on a zeroed tile so the PE
clock un-throttles (HAM needs ~3.4us of continuous activity) while the
first input DMAs land; warmup counts are tuned so warmup ends exactly
when the gating transfers arrive. All normalization (stage-1 sums into
newV, stage-2 sums into the output) happens on the host, off the
device critical path.
"""

import math
import time

import numpy as np

try:  # degrade tracing gracefully on images without the axon NTFF hook
    import antenv.axon_hooks  # noqa: F401
except Exception:
    import sys as _sys
    import types as _types

    _m = _types.ModuleType("antenv.axon_hooks")
    _m._h = None
    _m.set_axon_ntff_profile_hook = lambda h: setattr(_m, "_h", h)
    _m.get_axon_ntff_profile_hook = lambda: _m._h
    _sys.modules["antenv.axon_hooks"] = _m

    # Best-effort: drive NTFF profiling via ctypes against the axon PJRT
    # plugin (same ABI trn_boot uses) so traced runs report exec time.
    try:
        import contextlib as _ctx
        import ctypes as _ct

        _lib = _ct.CDLL("/opt/axon/libaxon_pjrt.so")
        _lib.axon_start_nrt_profile.argtypes = [_ct.POINTER(_ct.c_int64),
                                                _ct.c_size_t]
        _lib.axon_start_nrt_profile.restype = _ct.c_int64
        _lib.axon_stop_nrt_profile.argtypes = [_ct.c_char_p]
        _lib.axon_stop_nrt_profile.restype = _ct.c_int64

        @_ctx.contextmanager
        def _ntff_hook(output_dir, device_ids):
            import jax

            jax.devices()
            if device_ids:
                ids = (_ct.c_int64 * len(device_ids))(*device_ids)
                rc = _lib.axon_start_nrt_profile(ids, len(device_ids))
            else:
                rc = _lib.axon_start_nrt_profile(None, 0)
            if rc != 0:
                raise RuntimeError(f"axon_start_nrt_profile rc={rc}")
            try:
                yield
            finally:
                n = _lib.axon_stop_nrt_profile(str(output_dir).encode())
                print(f"profile: {n} ntff file(s) -> {output_dir}")

        if hasattr(_lib, "axon_start_nrt_profile"):
            _m._h = _ntff_hook
    except Exception:
        pass

import ml_dtypes

import concourse.bass as bass
import concourse.bass_utils as _bass_utils
import concourse.tile as tile
from concourse import bacc, bass_isa, mybir
from concourse.bass_utils import run_bass_kernel_spmd

if not getattr(_bass_utils, "_upload_guarded", False):
    _orig_upload = _bass_utils.upload_artifacts

    def _safe_upload(tmpdir):
        try:
            return _orig_upload(tmpdir)
        except Exception:
            return f"local://{tmpdir}"

    _bass_utils.upload_artifacts = _safe_upload
    _bass_utils._upload_guarded = True

F32 = mybir.dt.float32
BF16 = mybir.dt.bfloat16
EXP = mybir.ActivationFunctionType.Exp
COPYF = mybir.ActivationFunctionType.Copy
NPBF16 = ml_dtypes.bfloat16

B, FRAME, SFRAME, C, VC, H, W = 2, 9, 15, 128, 512, 40, 40
HW = H * W                      # 1600
MID = FRAME // 2                # 4
WK = SFRAME * HW                # 24000 support keys
NKT = (WK + 127) // 128         # 188 key tiles (last = 64 rows)
Q2 = FRAME * HW                 # 14400 stage-2 query columns per batch
NK2T = (HW + 127) // 128        # 13 stage-2 key tiles (last = 64 rows)
VE = VC + 2                     # stage-1 value rows carry 2 ones-columns

CC_WIDTHS = [512, 400, 400, 288]  # stage-1 column chunks
L2_OWN = Q2 // 4                # 3600 stage-2 columns per lane
L2_CHUNKS = [450] * 8           # uniform: no LDWEIGHTS-bound tail chunk
INV_SQRT_C = 1.0 / math.sqrt(C)

_cache = {}


FW = VE + 128                   # fused per-key-tile row: [svte row | skT col tile]
NKL = NKT // 4                  # 47 key tiles per lane (k-split data parallel)
N_PAD_ROWS = NKT * 128 - WK     # 64 zero-padded key rows (lane 3's last tile)
N_WARM1 = 22                    # PE warmup matmuls (HAM un-throttle), stage 1
N_WARM2 = 17                    # >=~3.4us continuous: HAM un-throttles only
                                # after ~3.4us of uninterrupted PE activity
N_FILL2 = 12                    # stage-2 filler matmuls: keep the PE (and
                                # the HAM clock) busy while nvt streams in
LA = 4                          # stage-1 score-matmul lookahead (tiles):
                                # emitted after each value pack, so the four
                                # score PSUM banks never carry a WAR stall
S_MAJOR_TAIL = 8                # stage-1 last-chunk tiles run value-major
# fus DMA split: fine-grained early so the PE never overtakes a transfer
FUS_GROUPS = [1, 1, 1, 1, 1, 2, 2, 2, 3, 3, 4, 5, 6, 7, 8]
GRP = 4                         # key tiles per csum accumulation group
FW2 = 128                       # stage-2 fused nvt tile width (per key tile)


def _emit_warmup(nc, cpool, ps_pool, ps_tag, ps_shape, n_warm):
    """Throwaway matmuls on a zeroed tile, independent of any input DMA:
    keep the PE busy from t~0 so the HAM clock gate opens while the
    first real inputs are still in flight."""
    w_t = cpool.tile([128, 256], BF16, name="warm_sb")
    nc.vector.memset(w_t[:], 0.0)
    w_ps = ps_pool.tile(ps_shape, F32, name="warm_ps", tag=ps_tag)
    for _ in range(n_warm):
        nc.tensor.matmul(w_ps[:, 0:256], w_t[:, 0:128], w_t[:, 0:256],
                         start=True, stop=True)
    return w_t


def _build_stage1():
    nc = bacc.Bacc("TRN2", target_bir_lowering=False, debug=False, num_devices=8)
    fus = nc.dram_tensor("fus", [128, NKL * FW], BF16, kind="ExternalInput").ap()
    q1 = nc.dram_tensor("q1", [C, HW], BF16, kind="ExternalInput").ap()
    nv = nc.dram_tensor("nv", [VC, HW], BF16, kind="ExternalOutput").ap()
    csum = nc.dram_tensor("csum", [2, HW], F32, kind="ExternalOutput").ap()

    with tile.TileContext(nc) as tc:
        with (
            tc.tile_pool(name="const", bufs=1) as cpool,
            tc.tile_pool(name="fus", bufs=1) as fupool,
            tc.tile_pool(name="p", bufs=10) as ppool,
            tc.tile_pool(name="pacc", bufs=8) as paccpool,
            tc.tile_pool(name="capool", bufs=2) as capool,
            tc.tile_pool(name="out", bufs=6) as opool,
            tc.tile_pool(name="ps_s", bufs=4, space="PSUM") as ps_s,
            tc.tile_pool(name="ps_m", bufs=1, space="PSUM") as ps_m,
        ):
            _emit_warmup(nc, cpool, ps_s, "s_ps", [128, 512], N_WARM1)

            # q1's first chunk leads the sync queue (it gates the first
            # matmul); tile 0 rides both queues (half the partitions
            # each) so the first matmul's gate is ~half a tile-transfer.
            fu_t = fupool.tile([128, NKL * FW], BF16)
            q1_t = cpool.tile([C, HW], BF16)
            nc.sync.dma_start(fu_t[0:64, 0:FW], fus[0:64, 0:FW])
            nc.gpsimd.dma_start(fu_t[64:128, 0:FW], fus[64:128, 0:FW])
            nc.sync.dma_start(q1_t[:, 0:CC_WIDTHS[0]], q1[:, 0:CC_WIDTHS[0]])

            # the lane's whole key slice stays resident, partition-major in
            # DRAM so one DMA moves many tiles with long descriptors.
            # Small groups first so early tiles land fast; groups
            # alternate sync/gpsimd.
            off = 1
            for gi, g in enumerate(FUS_GROUPS[1:]):
                c0, c1 = off * FW, (off + g) * FW
                eng = nc.sync if gi % 2 == 0 else nc.gpsimd
                eng.dma_start(fu_t[:, c0:c1], fus[:, c0:c1])
                off += g
            nc.gpsimd.dma_start(q1_t[:, CC_WIDTHS[0]:], q1[:, CC_WIDTHS[0]:])

            # csum matmuls run once per GROUP of 4 key tiles: the idle DVE
            # pre-accumulates the exp(S) tiles, and each group's csum is
            # deferred so the tensor engine never waits on the DVE chain.
            co = 0
            for cc, W1 in enumerate(CC_WIDTHS):
                last = cc == len(CC_WIDTHS) - 1
                m_ps = [ps_m.tile([128, 512], F32, name=f"m_ps{cc}_{s}",
                                  tag=f"m_ps{s}") for s in range(4)]
                groups = []
                p_tiles = [None] * NKL
                st = {"ca": None, "p_prev": None, "p_acc": None,
                      "ones_ap": None}

                def emit_s_exp(e, cc=cc, W1=W1, co=co, p_tiles=p_tiles,
                               groups=groups, st=st):
                    # score matmul + exp for key tile e (runs LA tiles
                    # ahead of e's value matmuls)
                    fo = e * FW
                    s_ps = ps_s.tile([128, 512], F32, name="s_ps", tag="s_ps")
                    nc.tensor.matmul(s_ps[:, :W1], fu_t[:, fo + VE:fo + FW],
                                     q1_t[:, co:co + W1],
                                     start=True, stop=True)
                    p_t = ppool.tile([128, 512], BF16, name="p_t", tag="p_t")
                    nc.scalar.activation(p_t[:, :W1], s_ps[:, :W1], EXP,
                                         scale=INV_SQRT_C)
                    p_tiles[e] = p_t
                    j = e % GRP
                    if j == 0:
                        st["p_prev"] = p_t
                    elif j == 1:
                        st["p_acc"] = paccpool.tile([128, 512], BF16,
                                                    name="p_acc", tag="p_acc")
                        nc.vector.tensor_add(st["p_acc"][:, :W1],
                                             st["p_prev"][:, :W1],
                                             p_t[:, :W1])
                    else:
                        nc.vector.tensor_add(st["p_acc"][:, :W1],
                                             st["p_acc"][:, :W1],
                                             p_t[:, :W1])
                    if j == GRP - 1 or e == NKL - 1:
                        groups.append(st["p_acc"])
                        # collapse all but the last group on the (idle) DVE;
                        # gpsimd later all-reduces the collapsed tile across
                        # partitions, replacing 11 of the 12 csum matmuls
                        ng = len(groups)
                        if ng == 2:
                            st["ca"] = capool.tile([128, 512], F32,
                                                   name=f"ca{cc}", tag="ca")
                            nc.vector.tensor_add(st["ca"][:, :W1],
                                                 groups[0][:, :W1],
                                                 groups[1][:, :W1])
                        elif ng >= 3:
                            nc.vector.tensor_add(st["ca"][:, :W1],
                                                 st["ca"][:, :W1],
                                                 groups[ng - 1][:, :W1])

                kt_tmaj = NKL - S_MAJOR_TAIL if last else NKL

                def emit_mm(kt, s, W1=W1):
                    fo = kt * FW
                    nc.tensor.matmul(
                        m_ps[s][:, :W1],
                        fu_t[:, fo + 2 + 128 * s:fo + 2 + 128 * (s + 1)],
                        p_tiles[kt][:, :W1],
                        start=(kt == 0), stop=(kt == NKL - 1))

                def emit_evac(s, W1=W1, co=co, cc=cc):
                    m_sb = opool.tile([128, 512], BF16, name=f"m_sb{cc}_{s}",
                                      tag="m_sb")
                    # evacuations on the DVE: the scalar engine runs ONLY
                    # the exp chain, so the next chunk's first exp (which
                    # gates its first value matmul) is never queued
                    # behind a PSUM copy
                    nc.vector.tensor_copy(m_sb[:, :W1], m_ps[s][:, :W1])
                    eng = nc.sync if (s % 2 == 0 or cc == 3) else nc.gpsimd
                    eng.dma_start(nv[128 * s:128 * (s + 1), co:co + W1],
                                  m_sb[:, :W1])

                # prime all four score banks (no bank hazards), then emit
                # each lookahead score AFTER tile kt's value pack: that
                # bank's previous reader is exp(kt), which must have
                # completed for the value pack anyway -- so the score
                # pipeline never stalls, including across chunk
                # boundaries
                for e in range(min(LA, NKL)):
                    emit_s_exp(e)
                for kt in range(kt_tmaj):
                    for s in range(4):
                        emit_mm(kt, s)
                    if kt + LA < NKL:
                        emit_s_exp(kt + LA)
                for e in range(kt_tmaj + LA, NKL):
                    emit_s_exp(e)  # tail tiles' scores+exps (value-major path)

                # csum is all-DVE + gpsimd partition reduce: with no
                # ones-matmul PSUM bank, the score pool holds 4 buffers,
                # which carries the score pipeline across chunk
                # boundaries without waiting on the exp chain tail
                par = capool.tile([128, 512], F32, name=f"par{cc}", tag="par")
                nc.gpsimd.partition_all_reduce(par[:, :W1],
                                               st["ca"][:, :W1],
                                               128, bass_isa.ReduceOp.add)
                if not last:
                    for s in range(4):
                        emit_evac(s)
                else:
                    # final tiles run value-major: each accumulator
                    # finishes one 8-matmul sweep apart, so its PSUM
                    # evacuation and output DMA overlap the next sweep
                    # instead of bunching on the launch tail
                    for s in range(4):
                        for kt in range(kt_tmaj, NKL):
                            emit_mm(kt, s)
                        emit_evac(s)
                nc.sync.dma_start(csum[0:1, co:co + W1], par[0:1, :W1])
                co += W1
    nc.compile()
    return nc


def _build_stage2():
    nc = bacc.Bacc("TRN2", target_bir_lowering=False, debug=False, num_devices=8)
    mk = nc.dram_tensor("mk", [C, HW], BF16, kind="ExternalInput").ap()
    qq = nc.dram_tensor("qq", [C, L2_OWN], BF16, kind="ExternalInput").ap()
    nvt = nc.dram_tensor("nvt", [128, NK2T * VC], BF16,
                         kind="ExternalInput").ap()
    out = nc.dram_tensor("out", [VC, L2_OWN], BF16, kind="ExternalOutput").ap()
    c2 = nc.dram_tensor("c2", [2, L2_OWN], F32, kind="ExternalOutput").ap()

    with tile.TileContext(nc) as tc:
        with (
            tc.tile_pool(name="const", bufs=1) as cpool,
            tc.tile_pool(name="nvt", bufs=1) as nvpool,
            tc.tile_pool(name="p2", bufs=28) as p2pool,
            tc.tile_pool(name="ob", bufs=8) as obpool,
            tc.tile_pool(name="ca2", bufs=2) as ca2pool,
            tc.tile_pool(name="ps_s", bufs=4, space="PSUM") as ps_s,
            tc.tile_pool(name="ps_o", bufs=1, space="PSUM") as ps_o,
        ):
            _emit_warmup(nc, cpool, ps_s, "s_ps", [128, 512], N_WARM2)

            # DMA priority order = first-use order, and the DMA engines
            # ramp slowly for the first few us, so the gating transfers
            # (mk tile 0 + qq chunk 0) are split by partitions across
            # BOTH queues; the early nvt tiles follow immediately (they
            # gate the first output matmuls ~2us after the score sweep).
            # nvt is partition-major in DRAM (long descriptors).
            # sync = hardware DGE (fast, low latency): carries the gating
            # pair whole plus the first nvt tiles. gpsimd = software DGE
            # (~1us extra per transfer): carries everything needed later.
            mk_t = cpool.tile([C, HW], BF16)
            qq_t = cpool.tile([C, L2_OWN], BF16)
            nvt_t = nvpool.tile([128, NK2T * VC], BF16)
            c0 = L2_CHUNKS[0]
            nc.sync.dma_start(mk_t[:, 0:128], mk[:, 0:128])
            nc.sync.dma_start(qq_t[:, 0:c0], qq[:, 0:c0])
            nc.gpsimd.dma_start(mk_t[:, 128:HW], mk[:, 128:HW])
            for t0, t1 in ((0, 2), (2, 4)):
                nc.sync.dma_start(nvt_t[:, t0 * VC:t1 * VC],
                                  nvt[:, t0 * VC:t1 * VC])
            for t0, t1 in ((4, 6), (6, 9), (9, 13)):
                nc.gpsimd.dma_start(nvt_t[:, t0 * VC:t1 * VC],
                                    nvt[:, t0 * VC:t1 * VC])
            nc.sync.dma_start(qq_t[:, c0:4 * 450], qq[:, c0:4 * 450])
            nc.gpsimd.dma_start(qq_t[:, 4 * 450:], qq[:, 4 * 450:])

            col = 0
            for ci, chunk in enumerate(L2_CHUNKS):
                # S2 + exp; the idle DVE accumulates exp tiles in groups of 4
                # so the column-sum contraction costs 4 adds, not 13
                p2 = []
                p2acc = []
                for t in range(NK2T):
                    kk = min(128, HW - t * 128)
                    s_ps = ps_s.tile([128, 512], F32, name="s_ps", tag="s_ps")
                    nc.tensor.matmul(s_ps[:kk, :chunk],
                                     mk_t[:, t * 128:t * 128 + kk],
                                     qq_t[:, col:col + chunk],
                                     start=True, stop=True)
                    p_t = p2pool.tile([128, 512], BF16, tag="p2")
                    nc.scalar.activation(p_t[:kk, :chunk], s_ps[:kk, :chunk],
                                         EXP, scale=INV_SQRT_C)
                    p2.append(p_t)
                    j = t % 4
                    if j == 1:
                        pa = p2pool.tile([128, 512], BF16, tag="p2a", name="pa",
                                         bufs=6)
                        nc.vector.tensor_add(pa[:kk, :chunk],
                                             p2[t - 1][:kk, :chunk],
                                             p_t[:kk, :chunk])
                        p2acc.append(pa)
                    elif j > 1:
                        nc.vector.tensor_add(p2acc[-1][:kk, :chunk],
                                             p2acc[-1][:kk, :chunk],
                                             p_t[:kk, :chunk])
                if ci == 0:
                    # filler matmuls into a bank the output phase reuses
                    # later anyway: the first nvt tiles land ~2.5us after
                    # the score sweep ends, and an idle PE would trigger
                    # a HAM re-throttle that halves the clock for ~3.4us
                    f_ps = ps_o.tile([128, 512], F32, name="f_ps", tag="o_ps3")
                    for _ in range(N_FILL2):
                        nc.tensor.matmul(f_ps[:, :chunk], mk_t[:, 0:128],
                                         qq_t[:, 0:chunk],
                                         start=True, stop=True)
                # column sums via DVE collapse + gpsimd partition
                # all-reduce -- no tensor-engine csum matmuls at all
                ca2 = ca2pool.tile([128, 512], F32, name="ca2", tag="ca2")
                nc.vector.tensor_add(ca2[:, :chunk], p2acc[0][:, :chunk],
                                     p2acc[1][:, :chunk])
                nc.vector.tensor_add(ca2[:, :chunk], ca2[:, :chunk],
                                     p2acc[2][:, :chunk])
                nc.vector.tensor_add(ca2[:64, :chunk], ca2[:64, :chunk],
                                     p2[12][:64, :chunk])
                par2 = ca2pool.tile([128, 512], F32, name="par2", tag="par2")
                nc.gpsimd.partition_all_reduce(par2[:, :chunk],
                                               ca2[:, :chunk], 128,
                                               bass_isa.ReduceOp.add)
                nc.sync.dma_start(c2[0:1, col:col + chunk],
                                  par2[0:1, :chunk])

                o_ps = [ps_o.tile([128, 512], F32, name=f"o_ps{v}", tag=f"o_ps{v}")
                        for v in range(4)]
                for t in range(NK2T):
                    kk = min(128, HW - t * 128)
                    for v in range(4):
                        nc.tensor.matmul(
                            o_ps[v][:, :chunk],
                            nvt_t[:kk, t * VC + 128 * v:t * VC + 128 * (v + 1)],
                            p2[t][:kk, :chunk],
                            start=(t == 0), stop=(t == NK2T - 1))

                # evacuate unnormalized (bf16); the host divides by the
                # column sums. All copies on the DVE so the scalar
                # engine's exp chain (which frees the score PSUM banks
                # and feeds the output matmuls) is never delayed.
                for v in range(4):
                    ob = obpool.tile([128, 512], BF16, name=f"ob{v}", tag="ob")
                    nc.vector.tensor_copy(ob[:, :chunk], o_ps[v][:, :chunk])
                    eng = nc.sync if v % 2 == 0 else nc.gpsimd
                    eng.dma_start(out[128 * v:128 * (v + 1), col:col + chunk],
                                  ob[:, :chunk])
                col += chunk
    nc.compile()
    return nc


def _run_with_retry(build_key, builder, in_maps):
    """Run a launch; on a transient device failure retry, rebuilding the
    program (fresh jit identity) on the second failure."""
    last = None
    for attempt in range(3):
        if build_key not in _cache:
            _cache[build_key] = builder()
        try:
            return run_bass_kernel_spmd(_cache[build_key], in_maps,
                                        list(range(8)))
        except Exception as e:  # device wedge / transient axon failure
            last = e
            time.sleep(3.0)
            if attempt >= 1:
                _cache.pop(build_key, None)
    raise last


def kernel(query_q, query_k, support_k, support_v):
    query_q = np.ascontiguousarray(query_q, dtype=np.float32)
    query_k = np.ascontiguousarray(query_k, dtype=np.float32)
    support_k = np.ascontiguousarray(support_k, dtype=np.float32)
    support_v = np.ascontiguousarray(support_v, dtype=np.float32)

    # ---- host layout prep ----
    # fused per-key-tile rows: [1, 1, sv.T row (VC) | skT column tile (128)]
    WKP = NKT * 128
    fus = np.zeros((B, NKT, 128, FW), np.float32)
    fus[:, :, :, 0:2] = 1.0
    svt_pad = np.zeros((B, WKP, VC), np.float32)
    svt_pad[:, :WK] = support_v.transpose(0, 1, 3, 4, 2).reshape(B, WK, VC)
    fus[:, :, :, 2:VE] = svt_pad.reshape(B, NKT, 128, VC)
    skt_pad = np.zeros((B, C, WKP), np.float32)
    skt_pad[:, :, :WK] = support_k.transpose(0, 2, 1, 3, 4).reshape(B, C, WK)
    fus[:, :, :, VE:] = skt_pad.reshape(B, C, NKT, 128).transpose(0, 2, 1, 3)
    fus = fus.astype(NPBF16)
    # per-(batch,lane) partition-major layout: [128, NKL*FW]
    fusl = fus.reshape(B, 4, NKL, 128, FW).transpose(0, 1, 3, 2, 4) \
              .reshape(B, 4, 128, NKL * FW)
    q1 = np.ascontiguousarray(query_q[:, MID].reshape(B, C, HW)).astype(NPBF16)
    l1_maps = []
    for core in range(8):
        b, lane = divmod(core, 4)
        l1_maps.append({
            "fus": np.ascontiguousarray(fusl[b, lane]),
            "q1": q1[b],
        })
    res1 = _run_with_retry("l1", _build_stage1, l1_maps)
    r1 = res1.results

    # reduce the per-lane partial sums; normalize by the stage-1 column
    # sums on the host (zero-padded key rows contributed exp(0)=1 each);
    # build newV^T partition-major in bf16
    nvt_pm = np.empty((B, 128, NK2T * VC), NPBF16)
    for b in range(B):
        nv = sum(r1[4 * b + lane]["nv"].astype(np.float64) for lane in range(4))
        cs = sum(r1[4 * b + lane]["csum"][0].astype(np.float64)
                 for lane in range(4)) - float(N_PAD_ROWS)
        nvtp = np.zeros((NK2T * 128, VC), np.float64)
        nvtp[:HW] = (nv / cs[None, :]).T
        nvt_pm[b] = nvtp.reshape(NK2T, 128, VC).transpose(1, 0, 2) \
                        .reshape(128, NK2T * VC).astype(NPBF16)

    # ---- stage 2 ----
    mk = query_k[:, MID].reshape(B, C, HW).astype(NPBF16)
    qq = query_q.transpose(0, 2, 1, 3, 4).reshape(B, C, Q2).astype(NPBF16)
    l2_maps = []
    for core in range(8):
        b, lane = divmod(core, 4)
        w = lane * L2_OWN
        l2_maps.append({
            "mk": mk[b],
            "qq": np.ascontiguousarray(qq[b][:, w:w + L2_OWN]),
            "nvt": nvt_pm[b],
        })
    res2 = _run_with_retry("l2", _build_stage2, l2_maps)
    r2 = res2.results
    _cache["last_exec_ns"] = [res1.exec_time_ns, res2.exec_time_ns]

    outv = np.empty((B, VC, Q2), np.float32)
    for core in range(8):
        b, lane = divmod(core, 4)
        w = lane * L2_OWN
        outv[b][:, w:w + L2_OWN] = \
            r2[core]["out"].astype(np.float32) / r2[core]["c2"][0:1]

    # outv[b][vc, q2], q2 = f*HW + h*W + w  ->  [B, F, VC, H, W]
    return np.ascontiguousarray(
        outv.reshape(B, VC, FRAME, H, W).transpose(0, 2, 1, 3, 4))


# revision 33
# speedup vs baseline: 1.0037x; 1.0037x over previous
"""Trainium2 Bass kernel for the two-stage DAN/MoVe attention module.

Computation (per batch b, C=128 channels):
  Stage 1:  S  = skT.T @ q1 / sqrt(C);  P  = softmax_k(S);  newV = sv @ P
  Stage 2:  S2 = mK.T @ qq / sqrt(C);   P2 = softmax_k2(S2); out = newV @ P2

Sharding: 8 cores = 2 batches x 4 lanes. Stage 1 splits the 24000 support
keys 4 ways (47 key tiles each); stage 2 splits the 14400 frame-query
columns 4 ways (3600 each). Two SPMD launches; the host reduces the
stage-1 partial sums, normalizes, and transposes between launches.

All big matmuls run in bf16 (both operands; fp32 PSUM accumulate).
Softmax skips max-subtraction (scores ~N(0,1); exp cannot overflow).
Zero-padded key rows contribute exactly exp(0)=1 each to the stage-1
column sums; the host subtracts that constant instead of masking on
device. Column sums come from DVE-accumulated exp tiles collapsed and
all-reduced across partitions on the (otherwise idle) gpsimd engine --
no tensor-engine csum matmuls at all, which frees a PSUM bank for a
fourth score buffer.

Stage 1 primes all four score PSUM banks, then emits each lookahead
score AFTER the value pack whose exp freed that bank: the in-order
tensor queue then never stalls on the scalar engine's exp chain (the
baseline lost ~21us to 58 such stalls) including across chunk
boundaries. Chunk widths are [512,400,400,288]: the wide first chunk
slows the key-tile sweep while the fused sv|skT slices stream in, and
no chunk is narrow enough to become LDWEIGHTS/sequencer-bound. The
last chunk's final 8 key tiles run value-major so the four
accumulators finish staggered and their evacuation DMAs overlap the
remaining sweeps instead of bunching on the launch tail.

Stage 2 loads newV^T in a partition-major fused layout (one SBUF tile,
long-descriptor DMAs, like stage 1's fus) with the DMA order matched
to first use: mk tile 0 and the first qq chunk gate the first matmuls,
the early nvt tiles gate the first output matmuls.  The baseline's
1028B-descriptor nvte loads landed ~9us in, stalling the tensor queue
3.3us and triggering a HAM re-throttle; filler matmuls now bridge the
unavoidable nvt wait so the clock never drops.  Uniform 450-wide
chunks avoid the sequencer-bound 240-wide tail chunk.  All PSUM
evacuations ride the DVE so the scalar engine runs only the exp chain.

Each launch opens with throwaway matmuls on a zeroed tile so the PE
clock un-throttles (HAM needs ~3.4us of continuous activity) while the
first input DMAs land; warmup counts are tuned so warmup ends exactly
when the gating transfers arrive. All normalization (stage-1 sums into
newV, stage-2 sums into the output) happens on the host, off the
device critical path.
"""

import math
import time

import numpy as np

try:  # degrade tracing gracefully on images without the axon NTFF hook
    import antenv.axon_hooks  # noqa: F401
except Exception:
    import sys as _sys
    import types as _types

    _m = _types.ModuleType("antenv.axon_hooks")
    _m._h = None
    _m.set_axon_ntff_profile_hook = lambda h: setattr(_m, "_h", h)
    _m.get_axon_ntff_profile_hook = lambda: _m._h
    _sys.modules["antenv.axon_hooks"] = _m

    # Best-effort: drive NTFF profiling via ctypes against the axon PJRT
    # plugin (same ABI trn_boot uses) so traced runs report exec time.
    try:
        import contextlib as _ctx
        import ctypes as _ct

        _lib = _ct.CDLL("/opt/axon/libaxon_pjrt.so")
        _lib.axon_start_nrt_profile.argtypes = [_ct.POINTER(_ct.c_int64),
                                                _ct.c_size_t]
        _lib.axon_start_nrt_profile.restype = _ct.c_int64
        _lib.axon_stop_nrt_profile.argtypes = [_ct.c_char_p]
        _lib.axon_stop_nrt_profile.restype = _ct.c_int64

        @_ctx.contextmanager
        def _ntff_hook(output_dir, device_ids):
            import jax

            jax.devices()
            if device_ids:
                ids = (_ct.c_int64 * len(device_ids))(*device_ids)
                rc = _lib.axon_start_nrt_profile(ids, len(device_ids))
            else:
                rc = _lib.axon_start_nrt_profile(None, 0)
            if rc != 0:
                raise RuntimeError(f"axon_start_nrt_profile rc={rc}")
            try:
                yield
            finally:
                n = _lib.axon_stop_nrt_profile(str(output_dir).encode())
                print(f"profile: {n} ntff file(s) -> {output_dir}")

        if hasattr(_lib, "axon_start_nrt_profile"):
            _m._h = _ntff_hook
    except Exception:
        pass

import ml_dtypes

import concourse.bass as bass
import concourse.bass_utils as _bass_utils
import concourse.tile as tile
from concourse import bacc, bass_isa, mybir
from concourse.bass_utils import run_bass_kernel_spmd

if not getattr(_bass_utils, "_upload_guarded", False):
    _orig_upload = _bass_utils.upload_artifacts

    def _safe_upload(tmpdir):
        try:
            return _orig_upload(tmpdir)
        except Exception:
            return f"local://{tmpdir}"

    _bass_utils.upload_artifacts = _safe_upload
    _bass_utils._upload_guarded = True

F32 = mybir.dt.float32
BF16 = mybir.dt.bfloat16
EXP = mybir.ActivationFunctionType.Exp
COPYF = mybir.ActivationFunctionType.Copy
NPBF16 = ml_dtypes.bfloat16

B, FRAME, SFRAME, C, VC, H, W = 2, 9, 15, 128, 512, 40, 40
HW = H * W                      # 1600
MID = FRAME // 2                # 4
WK = SFRAME * HW                # 24000 support keys
NKT = (WK + 127) // 128         # 188 key tiles (last = 64 rows)
Q2 = FRAME * HW                 # 14400 stage-2 query columns per batch
NK2T = (HW + 127) // 128        # 13 stage-2 key tiles (last = 64 rows)
VE = VC + 2                     # stage-1 value rows carry 2 ones-columns

CC_WIDTHS = [512, 364, 364, 360]  # stage-1 column chunks: all >=350
                                  # so no chunk hits the ~135ns
                                  # sequencer+LDWEIGHTS spacing floor
L2_OWN = Q2 // 4                # 3600 stage-2 columns per lane
L2_CHUNKS = [450] * 8           # uniform: no LDWEIGHTS-bound tail chunk
INV_SQRT_C = 1.0 / math.sqrt(C)

_cache = {}


FW = VE + 128                   # fused per-key-tile row: [svte row | skT col tile]
NKL = NKT // 4                  # 47 key tiles per lane (k-split data parallel)
N_PAD_ROWS = NKT * 128 - WK     # 64 zero-padded key rows (lane 3's last tile)
N_WARM1 = 22                    # PE warmup matmuls (HAM un-throttle), stage 1
N_WARM2 = 19                    # >=~3.4us continuous (HAM un-throttle) AND
                                # ends ~when the gating qq chunk lands
N_FILL2 = 12                    # stage-2 filler matmuls: keep the PE (and
                                # the HAM clock) busy while nvt streams in
LA = 4                          # stage-1 score-matmul lookahead (tiles):
                                # emitted after each value pack, so the four
                                # score PSUM banks never carry a WAR stall
S_MAJOR_TAIL = 8                # stage-1 last-chunk tiles run value-major
# fus DMA split: fine-grained early so the PE never overtakes a transfer
FUS_GROUPS = [1, 1, 1, 1, 1, 2, 2, 2, 3, 3, 4, 5, 6, 7, 8]
GRP = 4                         # key tiles per csum accumulation group
FW2 = 128                       # stage-2 fused nvt tile width (per key tile)


def _emit_warmup(nc, cpool, ps_pool, ps_tag, ps_shape, n_warm):
    """Throwaway matmuls on a zeroed tile, independent of any input DMA:
    keep the PE busy from t~0 so the HAM clock gate opens while the
    first real inputs are still in flight."""
    w_t = cpool.tile([128, 256], BF16, name="warm_sb")
    nc.vector.memset(w_t[:], 0.0)
    w_ps = ps_pool.tile(ps_shape, F32, name="warm_ps", tag=ps_tag)
    for _ in range(n_warm):
        nc.tensor.matmul(w_ps[:, 0:256], w_t[:, 0:128], w_t[:, 0:256],
                         start=True, stop=True)
    return w_t


def _build_stage1():
    nc = bacc.Bacc("TRN2", target_bir_lowering=False, debug=False, num_devices=8)
    fus = nc.dram_tensor("fus", [128, NKL * FW], BF16, kind="ExternalInput").ap()
    q1 = nc.dram_tensor("q1", [C, HW], BF16, kind="ExternalInput").ap()
    nv = nc.dram_tensor("nv", [VC, HW], BF16, kind="ExternalOutput").ap()
    csum = nc.dram_tensor("csum", [2, HW], F32, kind="ExternalOutput").ap()

    with tile.TileContext(nc) as tc:
        with (
            tc.tile_pool(name="const", bufs=1) as cpool,
            tc.tile_pool(name="fus", bufs=1) as fupool,
            tc.tile_pool(name="p", bufs=10) as ppool,
            tc.tile_pool(name="pacc", bufs=8) as paccpool,
            tc.tile_pool(name="capool", bufs=2) as capool,
            tc.tile_pool(name="out", bufs=6) as opool,
            tc.tile_pool(name="ps_s", bufs=4, space="PSUM") as ps_s,
            tc.tile_pool(name="ps_m", bufs=1, space="PSUM") as ps_m,
        ):
            _emit_warmup(nc, cpool, ps_s, "s_ps", [128, 512], N_WARM1)

            # q1's first chunk leads the sync queue (it gates the first
            # matmul); tile 0 rides both queues (half the partitions
            # each) so the first matmul's gate is ~half a tile-transfer.
            fu_t = fupool.tile([128, NKL * FW], BF16)
            q1_t = cpool.tile([C, HW], BF16)
            nc.sync.dma_start(fu_t[0:64, 0:FW], fus[0:64, 0:FW])
            nc.gpsimd.dma_start(fu_t[64:128, 0:FW], fus[64:128, 0:FW])
            nc.sync.dma_start(q1_t[:, 0:CC_WIDTHS[0]], q1[:, 0:CC_WIDTHS[0]])

            # the lane's whole key slice stays resident, partition-major in
            # DRAM so one DMA moves many tiles with long descriptors.
            # Small groups first so early tiles land fast; groups
            # alternate sync/gpsimd.
            off = 1
            for gi, g in enumerate(FUS_GROUPS[1:]):
                c0, c1 = off * FW, (off + g) * FW
                eng = nc.sync if gi % 2 == 0 else nc.gpsimd
                eng.dma_start(fu_t[:, c0:c1], fus[:, c0:c1])
                off += g
            nc.gpsimd.dma_start(q1_t[:, CC_WIDTHS[0]:], q1[:, CC_WIDTHS[0]:])

            # csum matmuls run once per GROUP of 4 key tiles: the idle DVE
            # pre-accumulates the exp(S) tiles, and each group's csum is
            # deferred so the tensor engine never waits on the DVE chain.
            # per-chunk score/exp emitters built up front so a chunk's
            # 4-score prime can be emitted inside the PREVIOUS chunk
            # (one tile before its end): the exp chain then has a ~2us
            # head start and the first value pack of each chunk never
            # waits on it
            chunk_ctx = []
            co = 0
            for cc, W1 in enumerate(CC_WIDTHS):
                chunk_ctx.append({"cc": cc, "W1": W1, "co": co,
                                  "groups": [], "p_tiles": [None] * NKL,
                                  "st": {"ca": None, "p_prev": None,
                                         "p_acc": None, "ones_ap": None}})
                co += W1

            def s_exp(ctx, e):
                # score matmul + exp for key tile e of chunk ctx (runs LA
                # tiles ahead of e's value matmuls)
                W1, co, cc = ctx["W1"], ctx["co"], ctx["cc"]
                p_tiles, groups, st = ctx["p_tiles"], ctx["groups"], ctx["st"]
                fo = e * FW
                s_ps = ps_s.tile([128, 512], F32, name="s_ps", tag="s_ps")
                nc.tensor.matmul(s_ps[:, :W1], fu_t[:, fo + VE:fo + FW],
                                 q1_t[:, co:co + W1],
                                 start=True, stop=True)
                p_t = ppool.tile([128, 512], BF16, name="p_t", tag="p_t")
                nc.scalar.activation(p_t[:, :W1], s_ps[:, :W1], EXP,
                                     scale=INV_SQRT_C)
                p_tiles[e] = p_t
                j = e % GRP
                if j == 0:
                    st["p_prev"] = p_t
                elif j == 1:
                    st["p_acc"] = paccpool.tile([128, 512], BF16,
                                                name="p_acc", tag="p_acc")
                    nc.vector.tensor_add(st["p_acc"][:, :W1],
                                         st["p_prev"][:, :W1],
                                         p_t[:, :W1])
                else:
                    nc.vector.tensor_add(st["p_acc"][:, :W1],
                                         st["p_acc"][:, :W1],
                                         p_t[:, :W1])
                if j == GRP - 1 or e == NKL - 1:
                    groups.append(st["p_acc"])
                    # collapse all but the last group on the (idle) DVE;
                    # gpsimd later all-reduces the collapsed tile across
                    # partitions, replacing 11 of the 12 csum matmuls
                    ng = len(groups)
                    if ng == 2:
                        st["ca"] = capool.tile([128, 512], F32,
                                               name=f"ca{cc}", tag="ca")
                        nc.vector.tensor_add(st["ca"][:, :W1],
                                             groups[0][:, :W1],
                                             groups[1][:, :W1])
                    elif ng >= 3:
                        nc.vector.tensor_add(st["ca"][:, :W1],
                                             st["ca"][:, :W1],
                                             groups[ng - 1][:, :W1])

            for cc, W1 in enumerate(CC_WIDTHS):
                ctx = chunk_ctx[cc]
                co = ctx["co"]
                last = cc == len(CC_WIDTHS) - 1
                m_ps = [ps_m.tile([128, 512], F32, name=f"m_ps{cc}_{s}",
                                  tag=f"m_ps{s}") for s in range(4)]
                p_tiles = ctx["p_tiles"]
                st = ctx["st"]

                def emit_s_exp(e, ctx=ctx):
                    s_exp(ctx, e)

                kt_tmaj = NKL - S_MAJOR_TAIL if last else NKL

                def emit_mm(kt, s, W1=W1):
                    fo = kt * FW
                    nc.tensor.matmul(
                        m_ps[s][:, :W1],
                        fu_t[:, fo + 2 + 128 * s:fo + 2 + 128 * (s + 1)],
                        p_tiles[kt][:, :W1],
                        start=(kt == 0), stop=(kt == NKL - 1))

                def emit_evac(s, W1=W1, co=co, cc=cc):
                    m_sb = opool.tile([128, 512], BF16, name=f"m_sb{cc}_{s}",
                                      tag="m_sb")
                    # evacuations on the DVE: the scalar engine runs ONLY
                    # the exp chain, so the next chunk's first exp (which
                    # gates its first value matmul) is never queued
                    # behind a PSUM copy
                    nc.vector.tensor_copy(m_sb[:, :W1], m_ps[s][:, :W1])
                    eng = nc.sync if (s % 2 == 0 or cc == 3) else nc.gpsimd
                    eng.dma_start(nv[128 * s:128 * (s + 1), co:co + W1],
                                  m_sb[:, :W1])

                # prime all four score banks (no bank hazards), then emit
                # each lookahead score AFTER tile kt's value pack: that
                # bank's previous reader is exp(kt), which must have
                # completed for the value pack anyway -- so the score
                # pipeline never stalls, including across chunk
                # boundaries
                if cc == 0:
                    for e in range(min(LA, NKL)):
                        emit_s_exp(e)
                # (for cc>0 the prime was emitted inside chunk cc-1)
                for kt in range(kt_tmaj):
                    for s in range(4):
                        emit_mm(kt, s)
                    if kt + LA < NKL:
                        emit_s_exp(kt + LA)
                    elif kt == NKL - 2 and not last:
                        for e2 in range(min(LA, NKL)):
                            s_exp(chunk_ctx[cc + 1], e2)
                for e in range(kt_tmaj + LA, NKL):
                    emit_s_exp(e)  # tail tiles' scores+exps (value-major path)

                # csum is all-DVE + gpsimd partition reduce: with no
                # ones-matmul PSUM bank, the score pool holds 4 buffers,
                # which carries the score pipeline across chunk
                # boundaries without waiting on the exp chain tail
                par = capool.tile([128, 512], F32, name=f"par{cc}", tag="par")
                nc.gpsimd.partition_all_reduce(par[:, :W1],
                                               st["ca"][:, :W1],
                                               128, bass_isa.ReduceOp.add)
                if not last:
                    for s in range(4):
                        emit_evac(s)
                else:
                    # final tiles run value-major: each accumulator
                    # finishes one 8-matmul sweep apart, so its PSUM
                    # evacuation and output DMA overlap the next sweep
                    # instead of bunching on the launch tail
                    for s in range(4):
                        for kt in range(kt_tmaj, NKL):
                            emit_mm(kt, s)
                        emit_evac(s)
                nc.sync.dma_start(csum[0:1, co:co + W1], par[0:1, :W1])
                co += W1
    nc.compile()
    return nc


def _build_stage2():
    nc = bacc.Bacc("TRN2", target_bir_lowering=False, debug=False, num_devices=8)
    mk = nc.dram_tensor("mk", [C, HW], BF16, kind="ExternalInput").ap()
    qq = nc.dram_tensor("qq", [C, L2_OWN], BF16, kind="ExternalInput").ap()
    nvt = nc.dram_tensor("nvt", [128, NK2T * VC], BF16,
                         kind="ExternalInput").ap()
    out = nc.dram_tensor("out", [VC, L2_OWN], BF16, kind="ExternalOutput").ap()
    c2 = nc.dram_tensor("c2", [2, L2_OWN], F32, kind="ExternalOutput").ap()

    with tile.TileContext(nc) as tc:
        with (
            tc.tile_pool(name="const", bufs=1) as cpool,
            tc.tile_pool(name="nvt", bufs=1) as nvpool,
            tc.tile_pool(name="p2", bufs=28) as p2pool,
            tc.tile_pool(name="ob", bufs=8) as obpool,
            tc.tile_pool(name="ca2", bufs=2) as ca2pool,
            tc.tile_pool(name="ps_s", bufs=4, space="PSUM") as ps_s,
            tc.tile_pool(name="ps_o", bufs=1, space="PSUM") as ps_o,
        ):
            _emit_warmup(nc, cpool, ps_s, "s_ps", [128, 512], N_WARM2)

            # DMA priority order = first-use order, and the DMA engines
            # ramp slowly for the first few us, so the gating transfers
            # (mk tile 0 + qq chunk 0) are split by partitions across
            # BOTH queues; the early nvt tiles follow immediately (they
            # gate the first output matmuls ~2us after the score sweep).
            # nvt is partition-major in DRAM (long descriptors).
            # sync = hardware DGE (fast, low latency): carries the gating
            # pair whole plus the first nvt tiles. gpsimd = software DGE
            # (~1us extra per transfer): carries everything needed later.
            mk_t = cpool.tile([C, HW], BF16)
            qq_t = cpool.tile([C, L2_OWN], BF16)
            nvt_t = nvpool.tile([128, NK2T * VC], BF16)
            c0 = L2_CHUNKS[0]
            nc.sync.dma_start(mk_t[:, 0:128], mk[:, 0:128])
            nc.sync.dma_start(qq_t[:, 0:c0], qq[:, 0:c0])
            nc.gpsimd.dma_start(mk_t[:, 128:HW], mk[:, 128:HW])
            for t0, t1 in ((0, 2), (2, 4)):
                nc.sync.dma_start(nvt_t[:, t0 * VC:t1 * VC],
                                  nvt[:, t0 * VC:t1 * VC])
            for t0, t1 in ((4, 6), (6, 9), (9, 13)):
                nc.gpsimd.dma_start(nvt_t[:, t0 * VC:t1 * VC],
                                    nvt[:, t0 * VC:t1 * VC])
            nc.sync.dma_start(qq_t[:, c0:4 * 450], qq[:, c0:4 * 450])
            nc.gpsimd.dma_start(qq_t[:, 4 * 450:], qq[:, 4 * 450:])

            col = 0
            for ci, chunk in enumerate(L2_CHUNKS):
                # S2 + exp; the idle DVE accumulates exp tiles in groups of 4
                # so the column-sum contraction costs 4 adds, not 13
                p2 = []
                p2acc = []
                for t in range(NK2T):
                    kk = min(128, HW - t * 128)
                    s_ps = ps_s.tile([128, 512], F32, name="s_ps", tag="s_ps")
                    nc.tensor.matmul(s_ps[:kk, :chunk],
                                     mk_t[:, t * 128:t * 128 + kk],
                                     qq_t[:, col:col + chunk],
                                     start=True, stop=True)
                    p_t = p2pool.tile([128, 512], BF16, tag="p2")
                    nc.scalar.activation(p_t[:kk, :chunk], s_ps[:kk, :chunk],
                                         EXP, scale=INV_SQRT_C)
                    p2.append(p_t)
                    j = t % 4
                    if j == 1:
                        pa = p2pool.tile([128, 512], BF16, tag="p2a", name="pa",
                                         bufs=6)
                        nc.vector.tensor_add(pa[:kk, :chunk],
                                             p2[t - 1][:kk, :chunk],
                                             p_t[:kk, :chunk])
                        p2acc.append(pa)
                    elif j > 1:
                        nc.vector.tensor_add(p2acc[-1][:kk, :chunk],
                                             p2acc[-1][:kk, :chunk],
                                             p_t[:kk, :chunk])
                if ci == 0:
                    # filler matmuls into a bank the output phase reuses
                    # later anyway: the first nvt tiles land ~2.5us after
                    # the score sweep ends, and an idle PE would trigger
                    # a HAM re-throttle that halves the clock for ~3.4us
                    f_ps = ps_o.tile([128, 512], F32, name="f_ps", tag="o_ps3")
                    for _ in range(N_FILL2):
                        nc.tensor.matmul(f_ps[:, :chunk], mk_t[:, 0:128],
                                         qq_t[:, 0:chunk],
                                         start=True, stop=True)
                # column sums via DVE collapse + gpsimd partition
                # all-reduce -- no tensor-engine csum matmuls at all
                ca2 = ca2pool.tile([128, 512], F32, name="ca2", tag="ca2")
                nc.vector.tensor_add(ca2[:, :chunk], p2acc[0][:, :chunk],
                                     p2acc[1][:, :chunk])
                nc.vector.tensor_add(ca2[:, :chunk], ca2[:, :chunk],
                                     p2acc[2][:, :chunk])
                nc.vector.tensor_add(ca2[:64, :chunk], ca2[:64, :chunk],
                                     p2[12][:64, :chunk])
                par2 = ca2pool.tile([128, 512], F32, name="par2", tag="par2")
                nc.gpsimd.partition_all_reduce(par2[:, :chunk],
                                               ca2[:, :chunk], 128,
                                               bass_isa.ReduceOp.add)
                nc.sync.dma_start(c2[0:1, col:col + chunk],
                                  par2[0:1, :chunk])

                o_ps = [ps_o.tile([128, 512], F32, name=f"o_ps{v}", tag=f"o_ps{v}")
                        for v in range(4)]
                for t in range(NK2T):
                    kk = min(128, HW - t * 128)
                    for v in range(4):
                        nc.tensor.matmul(
                            o_ps[v][:, :chunk],
                            nvt_t[:kk, t * VC + 128 * v:t * VC + 128 * (v + 1)],
                            p2[t][:kk, :chunk],
                            start=(t == 0), stop=(t == NK2T - 1))

                # evacuate unnormalized (bf16); the host divides by the
                # column sums. All copies on the DVE so the scalar
                # engine's exp chain (which frees the score PSUM banks
                # and feeds the output matmuls) is never delayed.
                for v in range(4):
                    ob = obpool.tile([128, 512], BF16, name=f"ob{v}", tag="ob")
                    nc.vector.tensor_copy(ob[:, :chunk], o_ps[v][:, :chunk])
                    eng = nc.sync if v % 2 == 0 else nc.gpsimd
                    eng.dma_start(out[128 * v:128 * (v + 1), col:col + chunk],
                                  ob[:, :chunk])
                col += chunk
    nc.compile()
    return nc


def _run_with_retry(build_key, builder, in_maps):
    """Run a launch; on a transient device failure retry, rebuilding the
    program (fresh jit identity) on the second failure."""
    last = None
    for attempt in range(3):
        if build_key not in _cache:
            _cache[build_key] = builder()
        try:
            return run_bass_kernel_spmd(_cache[build_key], in_maps,
                                        list(range(8)))
        except Exception as e:  # device wedge / transient axon failure
            last = e
            time.sleep(3.0)
            if attempt >= 1:
                _cache.pop(build_key, None)
    raise last


def kernel(query_q, query_k, support_k, support_v):
    query_q = np.ascontiguousarray(query_q, dtype=np.float32)
    query_k = np.ascontiguousarray(query_k, dtype=np.float32)
    support_k = np.ascontiguousarray(support_k, dtype=np.float32)
    support_v = np.ascontiguousarray(support_v, dtype=np.float32)

    # ---- host layout prep ----
    # fused per-key-tile rows: [1, 1, sv.T row (VC) | skT column tile (128)]
    WKP = NKT * 128
    fus = np.zeros((B, NKT, 128, FW), np.float32)
    fus[:, :, :, 0:2] = 1.0
    svt_pad = np.zeros((B, WKP, VC), np.float32)
    svt_pad[:, :WK] = support_v.transpose(0, 1, 3, 4, 2).reshape(B, WK, VC)
    fus[:, :, :, 2:VE] = svt_pad.reshape(B, NKT, 128, VC)
    skt_pad = np.zeros((B, C, WKP), np.float32)
    skt_pad[:, :, :WK] = support_k.transpose(0, 2, 1, 3, 4).reshape(B, C, WK)
    fus[:, :, :, VE:] = skt_pad.reshape(B, C, NKT, 128).transpose(0, 2, 1, 3)
    fus = fus.astype(NPBF16)
    # per-(batch,lane) partition-major layout: [128, NKL*FW]
    fusl = fus.reshape(B, 4, NKL, 128, FW).transpose(0, 1, 3, 2, 4) \
              .reshape(B, 4, 128, NKL * FW)
    q1 = np.ascontiguousarray(query_q[:, MID].reshape(B, C, HW)).astype(NPBF16)
    l1_maps = []
    for core in range(8):
        b, lane = divmod(core, 4)
        l1_maps.append({
            "fus": np.ascontiguousarray(fusl[b, lane]),
            "q1": q1[b],
        })
    res1 = _run_with_retry("l1", _build_stage1, l1_maps)
    r1 = res1.results

    # reduce the per-lane partial sums; normalize by the stage-1 column
    # sums on the host (zero-padded key rows contributed exp(0)=1 each);
    # build newV^T partition-major in bf16
    nvt_pm = np.empty((B, 128, NK2T * VC), NPBF16)
    for b in range(B):
        nv = sum(r1[4 * b + lane]["nv"].astype(np.float64) for lane in range(4))
        cs = sum(r1[4 * b + lane]["csum"][0].astype(np.float64)
                 for lane in range(4)) - float(N_PAD_ROWS)
        nvtp = np.zeros((NK2T * 128, VC), np.float64)
        nvtp[:HW] = (nv / cs[None, :]).T
        nvt_pm[b] = nvtp.reshape(NK2T, 128, VC).transpose(1, 0, 2) \
                        .reshape(128, NK2T * VC).astype(NPBF16)

    # ---- stage 2 ----
    mk = query_k[:, MID].reshape(B, C, HW).astype(NPBF16)
    qq = query_q.transpose(0, 2, 1, 3, 4).reshape(B, C, Q2).astype(NPBF16)
    l2_maps = []
    for core in range(8):
        b, lane = divmod(core, 4)
        w = lane * L2_OWN
        l2_maps.append({
            "mk": mk[b],
            "qq": np.ascontiguousarray(qq[b][:, w:w + L2_OWN]),
            "nvt": nvt_pm[b],
        })
    res2 = _run_with_retry("l2", _build_stage2, l2_maps)
    r2 = res2.results
    _cache["last_exec_ns"] = [res1.exec_time_ns, res2.exec_time_ns]

    outv = np.empty((B, VC, Q2), np.float32)
    for core in range(8):
        b, lane = divmod(core, 4)
        w = lane * L2_OWN
        outv[b][:, w:w + L2_OWN] = \
            r2[core]["out"].astype(np.float32) / r2[core]["c2"][0:1]

    # outv[b][vc, q2], q2 = f*HW + h*W + w  ->  [B, F, VC, H, W]
    return np.ascontiguousarray(
        outv.reshape(B, VC, FRAME, H, W).transpose(0, 2, 1, 3, 4))
